# revision 41
# baseline (speedup 1.0000x reference)
"""AUGRU Trainium2 kernel v6 — v3 chain + packed-constant startup.

Same batch/half layout as the original kernel (b = 8j+k strided across
cores; j = 2c+half interleaved halves stacked on partitions; all on-chip
tensors [128, cols<=256]).

v6 delta vs v3: the 16 serialized constant DMAs (6 weights x 2 halves +
4 biases, ~645ns each on the one sync queue; ~10.3us before the first
matmul) are packed into two blobs on the host -- wblob [128, 384] bf16
(six [Wg.T; Wg.T] tiles side by side) and bblob [128, 4] f32 -- loaded
with two DMAs ordered ahead of the pair-0 x/w loads. The steady-state
loop's instruction stream is unchanged (weight/bias APs are views into
the blob tiles), saving ~8us of startup.

Chain shaves vs the original baseline (from v3):
  - psum gate outputs (nh, ni) are evacuated to bf16 SBUF off the critical
    path (nhb = nh + bhn via tensor_scalar_add; nib = copy), so the on-chain
    m1/a1 become 2x-mode bf16 tensor ops (~165ns instead of 351/380).
  - state is bf16 end-to-end: single h' sub, no f32 shadow, output DMA'd
    as bf16 from the h-state pair tiles (host masks + converts).
  - attention weights w are host-broadcast to [128, HALF] bf16 and DMA'd
    (no GpSimd DIRECT2D); zw multiply runs on GpSimd off-chain.
  - x-side matmuls sit ahead of the h-side ones in the PE queue, so they
    execute during the previous step's elementwise tail.
"""

import os
import ml_dtypes
import numpy as np

import concourse.bass as bass
import concourse.bacc as bacc
import concourse.mybir as mybir
from concourse.tile import TileContext
from concourse.bass_utils import run_bass_kernel_spmd

T, B, D, H = 200, 4096, 64, 64
NCORES = 8
BC = B // NCORES  # 512 batch rows per core
HALF = BC // 2    # 256 columns per half
P = T // 2        # step pairs

LAST_RESULT = None

f32 = mybir.dt.float32
bf16 = mybir.dt.bfloat16
AF = mybir.ActivationFunctionType
ALU = mybir.AluOpType


def _build_program(N1s, N2s):
    nc = bacc.Bacc()

    x_d = nc.declare_dram_parameter("x", [P, 128, 2, HALF], bf16, isOutput=False)
    w_d = nc.declare_dram_parameter("w", [P, 128, 2, HALF], bf16, isOutput=False)
    # all six weight tiles packed [128, 6*64] (halves pre-stacked on the
    # host) + the four biases packed [128, 4]: two startup DMAs, not 16
    wblob_d = nc.declare_dram_parameter("wblob", [128, 9 * H], bf16,
                                        isOutput=False)
    bblob_d = nc.declare_dram_parameter("bblob", [128, 4], f32, isOutput=False)
    op_d = nc.declare_dram_parameter("op", [P + 1, 128, 2, HALF], bf16,
                                     isOutput=True)

    with TileContext(nc) as tc:
        with (
            tc.tile_pool(name="const", bufs=1) as cpool,
            tc.tile_pool(name="hb", bufs=3) as hbpool,
            tc.tile_pool(name="xin", bufs=3) as xpool,
            tc.tile_pool(name="win", bufs=3) as wpool,
            tc.tile_pool(name="work", bufs=2) as spool,
            tc.tile_pool(name="ps", bufs=2, space="PSUM") as ppool,
        ):
            wv0 = int(N1s[0])
            hb_cur = hbpool.tile([128, 2, HALF], bf16, tag="hb")
            nc.vector.memset(hb_cur[:, 0, :], 0.0)
            wtile = cpool.tile([128, 9 * H], bf16, tag="wblob")
            nc.sync.dma_start(out=wtile[:, :], in_=wblob_d[:, :])
            x_cur = xpool.tile([128, 2, HALF], bf16, tag="x")
            nc.sync.dma_start(out=x_cur[:, :, 0:wv0], in_=x_d[0, :, :, 0:wv0])
            pw_cur = wpool.tile([128, 2, HALF], bf16, tag="pw")
            nc.sync.dma_start(out=pw_cur[:, :, 0:wv0], in_=w_d[0, :, :, 0:wv0])
            hb_nxt = x_nxt = pw_nxt = None

            btile = cpool.tile([128, 4], f32, tag="bblob")
            nc.sync.dma_start(out=btile[:, :], in_=bblob_d[:, :])
            wts = {
                name: wtile[:, i * H:(i + 1) * H]
                for i, name in enumerate(
                    ["wrx", "wzx", "wnx", "wrh", "wzh", "wnh",
                     "vrh", "vzh", "vnh"])
            }
            biases = {
                name: btile[:, i:i + 1]
                for i, name in enumerate(["br", "bz", "bhn", "bin"])
            }

            for t in range(T):
                N1 = int(N1s[t])
                N2 = int(N2s[t])
                if N1 == 0:
                    break
                p, s = divmod(t, 2)

                if s == 0:
                    hb_nxt = hbpool.tile([128, 2, HALF], bf16, tag="hb")
                    x_nxt = xpool.tile([128, 2, HALF], bf16, tag="x")
                    pw_nxt = wpool.tile([128, 2, HALF], bf16, tag="pw")
                    if p + 1 < P:
                        wvn = int(N1s[2 * (p + 1)])
                        if wvn > 0:
                            nc.sync.dma_start(out=x_nxt[:, :, 0:wvn],
                                              in_=x_d[p + 1, :, :, 0:wvn])
                            nc.sync.dma_start(out=pw_nxt[:, :, 0:wvn],
                                              in_=w_d[p + 1, :, :, 0:wvn])

                pr = ppool.tile([128, HALF], f32, tag="pr")
                pz = ppool.tile([128, HALF], f32, tag="pz")
                # pni/pnh as SEPARATE tiles (not two regions of one tile):
                # dependency tracking is tile-granular, so a shared tile made
                # nib wait on the pnh h-side matmuls and blocked m1's slot.
                pni = ppool.tile([128, HALF], f32, tag="pni")
                pnh = ppool.tile([128, HALF], f32, tag="pnh")

                def xmm(psum, wx, stop):
                    nc.tensor.matmul(psum[0:64, 0:N1], lhsT=wts[wx][0:64, :],
                                     rhs=x_cur[0:64, s, 0:N1], start=True,
                                     stop=stop)
                    if N2 > 0:
                        nc.tensor.matmul(psum[64:128, 0:N2],
                                         lhsT=wts[wx][64:128, :],
                                         rhs=x_cur[64:128, s, 0:N2],
                                         start=True, stop=stop)

                def hmm(psum, wh, start):
                    nc.tensor.matmul(psum[0:64, 0:N1], lhsT=wts[wh][0:64, :],
                                     rhs=hb_cur[0:64, s, 0:N1], start=start,
                                     stop=True)
                    if N2 > 0:
                        nc.tensor.matmul(psum[64:128, 0:N2],
                                         lhsT=wts[wh][64:128, :],
                                         rhs=hb_cur[64:128, s, 0:N2],
                                         start=start, stop=True)

                def umm(psum, wh, start, stop):
                    nc.tensor.matmul(psum[0:64, 0:N1], lhsT=wts[wh][0:64, :],
                                     rhs=uu_prev[0:64, 0:N1], start=start,
                                     stop=stop)
                    if N2 > 0:
                        nc.tensor.matmul(psum[64:128, 0:N2],
                                         lhsT=wts[wh][64:128, :],
                                         rhs=uu_prev[64:128, 0:N2],
                                         start=start, stop=stop)

                def vmm(psum, wh, start):
                    nc.tensor.matmul(psum[0:64, 0:N1], lhsT=wts[wh][0:64, :],
                                     rhs=vv_prev[0:64, 0:N1], start=start,
                                     stop=False)
                    if N2 > 0:
                        nc.tensor.matmul(psum[64:128, 0:N2],
                                         lhsT=wts[wh][64:128, :],
                                         rhs=vv_prev[64:128, 0:N2],
                                         start=start, stop=False)

                # x-side first: these run during the previous step's tail
                xmm(pr, "wrx", False)
                xmm(pz, "wzx", False)
                xmm(pni, "wnx", True)
                # h-side on (uu, vv) of the previous step: h = uu - vv is
                # distributed over the matmuls (negated v-side weights), so
                # `sub` leaves the critical cycle. vv lands before uu
                # (measured), so the v-pairs stream on the PE just ahead of
                # the u-pairs; u-side keeps pnh-first for the nhb path.
                # hybrid: at large N the z-path delivers vv after uu, so the
                # v-pairs would serialize into the head — use the direct-h
                # route there (sub on cycle); the split route wins below.
                if t == 0 or N1 >= 180:
                    # pr-first here: the v11 all-steps experiment lost
                    # overall but its band profile shows pr-first is ~87ns
                    # faster per step in this large-N regime.
                    hmm(pr, "wrh", False)
                    hmm(pnh, "wnh", True)
                    hmm(pz, "wzh", False)
                else:
                    vmm(pnh, "vnh", True)
                    vmm(pr, "vrh", False)
                    vmm(pz, "vzh", False)
                    umm(pnh, "wnh", False, True)
                    umm(pr, "wrh", False, True)
                    umm(pz, "wzh", False, True)

                # scalar engine: sigmoid(r) leads; sigmoid(z) fills the gap
                rs = spool.tile([128, HALF], bf16, tag="rs")
                nc.scalar.activation(rs[:, 0:N1], pr[:, 0:N1], AF.Sigmoid,
                                     bias=biases["br"][:, 0:1], scale=1.0)
                zs = spool.tile([128, HALF], bf16, tag="zs")
                nc.scalar.activation(zs[:, 0:N1], pz[:, 0:N1], AF.Sigmoid,
                                     bias=biases["bz"][:, 0:1], scale=1.0)

                # psum evacuations (off-chain, bf16) + fast on-chain m1/a1
                nhb = spool.tile([128, HALF], bf16, tag="nhb")
                nc.vector.tensor_scalar_add(nhb[:, 0:N1], pnh[:, 0:N1],
                                            biases["bhn"][:, 0:1])
                m1 = spool.tile([128, HALF], bf16, tag="m1")
                nc.vector.tensor_mul(m1[:, 0:N1], rs[:, 0:N1], nhb[:, 0:N1])
                # nib: evacuate pni to bf16 so the on-cycle a1 is a 2x-mode
                # bf16 op instead of a 1x psum read (~120ns cheaper)
                # nib split in two half-width ops: it runs ~2 steps early in
                # whatever vector idle gap the greedy scheduler finds, and a
                # full-width op wedged into the zw->vv gap was stalling uu.
                nib = spool.tile([128, HALF], bf16, tag="nib")
                nh1 = (N1 + 1) // 2 if N1 >= 150 else N1
                nc.vector.tensor_scalar_add(nib[:, 0:nh1], pni[:, 0:nh1], 0.0)
                if nh1 < N1:
                    nc.vector.tensor_scalar_add(nib[:, nh1:N1],
                                                pni[:, nh1:N1], 0.0)
                a1 = spool.tile([128, HALF], bf16, tag="a1")
                nc.vector.tensor_add(a1[:, 0:N1], m1[:, 0:N1], nib[:, 0:N1])
                nt = spool.tile([128, HALF], bf16, tag="nt")
                nc.scalar.activation(nt[:, 0:N1], a1[:, 0:N1], AF.Tanh,
                                     bias=biases["bin"][:, 0:1], scale=1.0)

                # z path off-chain: zw, then vv = (zw-1) (.) h. At large N
                # the GpSimd zw (~2ns/col) lands so late that vv overruns
                # into uu's vector slot (measured the->uu stalls); run zw on
                # the vector engine there instead (~0.5ns/col, fits the
                # a1->tanh idle window). Small N keeps zw on GpSimd to keep
                # the vector queue light.
                zw = spool.tile([128, HALF], bf16, tag="zw")
                if N1 >= 110:
                    nc.vector.tensor_mul(zw[:, 0:N1], zs[:, 0:N1],
                                         pw_cur[:, s, 0:N1])
                else:
                    nc.gpsimd.tensor_mul(zw[:, 0:N1], zs[:, 0:N1],
                                         pw_cur[:, s, 0:N1])
                vv = spool.tile([128, HALF], bf16, tag="vv")
                nc.vector.scalar_tensor_tensor(
                    out=vv[:, 0:N1], in0=zw[:, 0:N1], scalar=1.0,
                    in1=hb_cur[:, s, 0:N1], op0=ALU.subtract, op1=ALU.mult)

                uu = spool.tile([128, HALF], bf16, tag="uu")
                nc.vector.tensor_mul(uu[:, 0:N1], zw[:, 0:N1], nt[:, 0:N1])
                hdst = hb_cur if s == 0 else hb_nxt
                nc.vector.tensor_sub(hdst[:, 1 - s, 0:N1], uu[:, 0:N1],
                                     vv[:, 0:N1])

                if s == 0:
                    wide = int(N1s[2 * p - 1]) if p > 0 else int(N1s[0])
                    nc.sync.dma_start(out=op_d[p, :, 0:2, 0:wide],
                                      in_=hb_cur[:, 0:2, 0:wide])
                if s == 1:
                    hb_cur, x_cur, pw_cur = hb_nxt, x_nxt, pw_nxt
                uu_prev, vv_prev = uu, vv

            vlast = int(N1s[T - 1])
            nc.sync.dma_start(out=op_d[P, :, 0:1, 0:vlast],
                              in_=hb_cur[:, 0:1, 0:vlast])

    nc.compile()
    return nc


def kernel(x, weights, lengths, W_ih, W_hh, b_ih, b_hh):
    global LAST_RESULT
    x = np.asarray(x, dtype=np.float32)
    weights = np.asarray(weights, dtype=np.float32)
    lengths = np.asarray(lengths, dtype=np.int32)
    W_ih = np.asarray(W_ih, dtype=np.float32)
    W_hh = np.asarray(W_hh, dtype=np.float32)
    b_ih = np.asarray(b_ih, dtype=np.float32)
    b_hh = np.asarray(b_hh, dtype=np.float32)

    counts = (lengths[None, :] > np.arange(T)[:, None]).sum(axis=1)  # [T]
    v = -(-counts // NCORES)
    N1s = (v + 1) // 2
    N2s = v // 2

    # x: [T,B,D] -> per-core [P, 128, 2, HALF] (feat-major, halves stacked,
    # two steps per block) — identical to the original kernel's layout
    xr = x.reshape(T, BC, NCORES, D)            # [t, j, k, d], b = 8j+k
    xr = xr.transpose(2, 0, 3, 1)               # [k, t, d, j]
    xr = xr.reshape(NCORES, T, D, HALF, 2)      # j = 2c + half
    xr = xr.transpose(0, 1, 4, 2, 3)            # [k, t, half, d, c]
    x8 = xr.reshape(NCORES, T, 128, HALF).astype(ml_dtypes.bfloat16)
    x8 = np.ascontiguousarray(
        x8.reshape(NCORES, P, 2, 128, HALF).transpose(0, 1, 3, 2, 4))

    wr = weights[:, :, 0].reshape(T, BC, NCORES)   # [t, j, k]
    wr = wr.transpose(2, 0, 1)                     # [k, t, j]
    wr = wr.reshape(NCORES, T, HALF, 2).transpose(0, 1, 3, 2)  # [k,t,half,c]
    w8 = np.broadcast_to(wr[:, :, :, None, :],
                         (NCORES, T, 2, 64, HALF))  # [k, t, half, d, c]
    w8 = w8.reshape(NCORES, T, 128, HALF)
    w8 = np.ascontiguousarray(
        w8.reshape(NCORES, P, 2, 128, HALF).transpose(0, 1, 3, 2, 4)
    ).astype(ml_dtypes.bfloat16)

    # wblob: six [128, 64] tiles (each = [Wg.T; Wg.T] halves-stacked) packed
    # side by side; bblob: the four [128, 1] biases packed side by side.
    wtiles = [np.vstack([wg.T, wg.T]) for wg in
              (W_ih[0:64], W_ih[64:128], W_ih[128:192],
               W_hh[0:64], W_hh[64:128], W_hh[128:192],
               -W_hh[0:64], -W_hh[64:128], -W_hh[128:192])]
    wblob = np.ascontiguousarray(
        np.concatenate(wtiles, axis=1)).astype(ml_dtypes.bfloat16)
    b_r = (b_ih[0:64] + b_hh[0:64]).astype(np.float32)
    b_z = (b_ih[64:128] + b_hh[64:128]).astype(np.float32)
    b_hn = b_hh[128:192].astype(np.float32)
    b_in = b_ih[128:192].astype(np.float32)
    bblob = np.stack([np.tile(b_r, 2), np.tile(b_z, 2),
                      np.tile(b_hn, 2), np.tile(b_in, 2)],
                     axis=1).astype(np.float32)  # [128, 4]
    nc = _build_program(N1s, N2s)

    in_maps = []
    for k in range(NCORES):
        m = {"x": x8[k], "w": w8[k], "wblob": wblob, "bblob": bblob}
        in_maps.append(m)

    trace = bool(os.environ.get("AUGRU_TRACE"))
    tmpdir = os.environ.get("AUGRU_PROF_DIR") or None
    res = run_bass_kernel_spmd(nc, in_maps, list(range(NCORES)), trace=trace,
                               tmpdir=tmpdir)
    LAST_RESULT = res

    # op: [k, P+1, 128, 2, HALF]; row t' = 2p+s holds h_{t'} = out[t'-1]
    outs = np.stack([np.asarray(res.results[k]["op"]) for k in range(NCORES)])
    o = outs.astype(np.float32)                       # [k, p, 128, s, c]
    o = o.transpose(1, 3, 0, 2, 4)                    # [p, s, k, 128, c]
    o = o.reshape(2 * (P + 1), NCORES, 2, H, HALF)    # [t', k, half, d, c]
    o = o.transpose(0, 4, 2, 1, 3)                    # [t', c, half, k, d]
    o = o.reshape(2 * (P + 1), B, H)[1:T + 1]         # j = 2c+half, b = 8j+k
    mask = (np.arange(T)[:, None] < lengths[None, :])
    o = np.where(mask[:, :, None], o, np.float32(0.0)).astype(np.float32)
    return o



# revision 43
# speedup vs baseline: 1.0100x; 1.0100x over previous
"""AUGRU Trainium2 kernel v6 — v3 chain + packed-constant startup.

Same batch/half layout as the original kernel (b = 8j+k strided across
cores; j = 2c+half interleaved halves stacked on partitions; all on-chip
tensors [128, cols<=256]).

v6 delta vs v3: the 16 serialized constant DMAs (6 weights x 2 halves +
4 biases, ~645ns each on the one sync queue; ~10.3us before the first
matmul) are packed into two blobs on the host -- wblob [128, 384] bf16
(six [Wg.T; Wg.T] tiles side by side) and bblob [128, 4] f32 -- loaded
with two DMAs ordered ahead of the pair-0 x/w loads. The steady-state
loop's instruction stream is unchanged (weight/bias APs are views into
the blob tiles), saving ~8us of startup.

Chain shaves vs the original baseline (from v3):
  - psum gate outputs (nh, ni) are evacuated to bf16 SBUF off the critical
    path (nhb = nh + bhn via tensor_scalar_add; nib = copy), so the on-chain
    m1/a1 become 2x-mode bf16 tensor ops (~165ns instead of 351/380).
  - state is bf16 end-to-end: single h' sub, no f32 shadow, output DMA'd
    as bf16 from the h-state pair tiles (host masks + converts).
  - attention weights w are host-broadcast to [128, HALF] bf16 and DMA'd
    (no GpSimd DIRECT2D); zw multiply runs on GpSimd off-chain.
  - x-side matmuls sit ahead of the h-side ones in the PE queue, so they
    execute during the previous step's elementwise tail.
"""

import os
import ml_dtypes
import numpy as np

import concourse.bass as bass
import concourse.bacc as bacc
import concourse.mybir as mybir
from concourse.tile import TileContext
from concourse.bass_utils import run_bass_kernel_spmd

T, B, D, H = 200, 4096, 64, 64
NCORES = 8
BC = B // NCORES  # 512 batch rows per core
HALF = BC // 2    # 256 columns per half
P = T // 2        # step pairs

LAST_RESULT = None

f32 = mybir.dt.float32
bf16 = mybir.dt.bfloat16
AF = mybir.ActivationFunctionType
ALU = mybir.AluOpType


def _build_program(N1s, N2s):
    nc = bacc.Bacc()

    x_d = nc.declare_dram_parameter("x", [P, 128, 2, HALF], bf16, isOutput=False)
    w_d = nc.declare_dram_parameter("w", [P, 128, 2, HALF], bf16, isOutput=False)
    # all six weight tiles packed [128, 6*64] (halves pre-stacked on the
    # host) + the four biases packed [128, 4]: two startup DMAs, not 16
    wblob_d = nc.declare_dram_parameter("wblob", [128, 9 * H], bf16,
                                        isOutput=False)
    bblob_d = nc.declare_dram_parameter("bblob", [128, 4], f32, isOutput=False)
    op_d = nc.declare_dram_parameter("op", [P + 1, 128, 2, HALF], bf16,
                                     isOutput=True)

    with TileContext(nc) as tc:
        with (
            tc.tile_pool(name="const", bufs=1) as cpool,
            tc.tile_pool(name="hb", bufs=3) as hbpool,
            tc.tile_pool(name="xin", bufs=3) as xpool,
            tc.tile_pool(name="win", bufs=3) as wpool,
            tc.tile_pool(name="work", bufs=2) as spool,
            tc.tile_pool(name="ps", bufs=2, space="PSUM") as ppool,
        ):
            wv0 = int(N1s[0])
            hb_cur = hbpool.tile([128, 2, HALF], bf16, tag="hb")
            nc.vector.memset(hb_cur[:, 0, :], 0.0)
            wtile = cpool.tile([128, 9 * H], bf16, tag="wblob")
            nc.sync.dma_start(out=wtile[:, :], in_=wblob_d[:, :])
            x_cur = xpool.tile([128, 2, HALF], bf16, tag="x")
            nc.sync.dma_start(out=x_cur[:, :, 0:wv0], in_=x_d[0, :, :, 0:wv0])
            pw_cur = wpool.tile([128, 2, HALF], bf16, tag="pw")
            nc.sync.dma_start(out=pw_cur[:, :, 0:wv0], in_=w_d[0, :, :, 0:wv0])
            hb_nxt = x_nxt = pw_nxt = None

            btile = cpool.tile([128, 4], f32, tag="bblob")
            nc.sync.dma_start(out=btile[:, :], in_=bblob_d[:, :])
            wts = {
                name: wtile[:, i * H:(i + 1) * H]
                for i, name in enumerate(
                    ["wrx", "wzx", "wnx", "wrh", "wzh", "wnh",
                     "vrh", "vzh", "vnh"])
            }
            biases = {
                name: btile[:, i:i + 1]
                for i, name in enumerate(["br", "bz", "bhn", "bin"])
            }

            for t in range(T):
                N1 = int(N1s[t])
                N2 = int(N2s[t])
                if N1 == 0:
                    break
                p, s = divmod(t, 2)

                if s == 0:
                    hb_nxt = hbpool.tile([128, 2, HALF], bf16, tag="hb")
                    x_nxt = xpool.tile([128, 2, HALF], bf16, tag="x")
                    pw_nxt = wpool.tile([128, 2, HALF], bf16, tag="pw")
                    if p + 1 < P:
                        wvn = int(N1s[2 * (p + 1)])
                        if wvn > 0:
                            nc.sync.dma_start(out=x_nxt[:, :, 0:wvn],
                                              in_=x_d[p + 1, :, :, 0:wvn])
                            nc.sync.dma_start(out=pw_nxt[:, :, 0:wvn],
                                              in_=w_d[p + 1, :, :, 0:wvn])

                pr = ppool.tile([128, HALF], f32, tag="pr")
                pz = ppool.tile([128, HALF], f32, tag="pz")
                # pni/pnh as SEPARATE tiles (not two regions of one tile):
                # dependency tracking is tile-granular, so a shared tile made
                # nib wait on the pnh h-side matmuls and blocked m1's slot.
                pni = ppool.tile([128, HALF], f32, tag="pni")
                pnh = ppool.tile([128, HALF], f32, tag="pnh")

                def xmm(psum, wx, stop):
                    nc.tensor.matmul(psum[0:64, 0:N1], lhsT=wts[wx][0:64, :],
                                     rhs=x_cur[0:64, s, 0:N1], start=True,
                                     stop=stop)
                    if N2 > 0:
                        nc.tensor.matmul(psum[64:128, 0:N2],
                                         lhsT=wts[wx][64:128, :],
                                         rhs=x_cur[64:128, s, 0:N2],
                                         start=True, stop=stop)

                def hmm(psum, wh, start):
                    nc.tensor.matmul(psum[0:64, 0:N1], lhsT=wts[wh][0:64, :],
                                     rhs=hb_cur[0:64, s, 0:N1], start=start,
                                     stop=True)
                    if N2 > 0:
                        nc.tensor.matmul(psum[64:128, 0:N2],
                                         lhsT=wts[wh][64:128, :],
                                         rhs=hb_cur[64:128, s, 0:N2],
                                         start=start, stop=True)

                def umm(psum, wh, start, stop):
                    nc.tensor.matmul(psum[0:64, 0:N1], lhsT=wts[wh][0:64, :],
                                     rhs=uu_prev[0:64, 0:N1], start=start,
                                     stop=stop)
                    if N2 > 0:
                        nc.tensor.matmul(psum[64:128, 0:N2],
                                         lhsT=wts[wh][64:128, :],
                                         rhs=uu_prev[64:128, 0:N2],
                                         start=start, stop=stop)

                def vmm(psum, wh, start):
                    nc.tensor.matmul(psum[0:64, 0:N1], lhsT=wts[wh][0:64, :],
                                     rhs=vv_prev[0:64, 0:N1], start=start,
                                     stop=False)
                    if N2 > 0:
                        nc.tensor.matmul(psum[64:128, 0:N2],
                                         lhsT=wts[wh][64:128, :],
                                         rhs=vv_prev[64:128, 0:N2],
                                         start=start, stop=False)

                # x-side first: these run during the previous step's tail
                xmm(pr, "wrx", False)
                xmm(pz, "wzx", False)
                xmm(pni, "wnx", True)
                # h-side on (uu, vv) of the previous step: h = uu - vv is
                # distributed over the matmuls (negated v-side weights), so
                # `sub` leaves the critical cycle. vv lands before uu
                # (measured), so the v-pairs stream on the PE just ahead of
                # the u-pairs; u-side keeps pnh-first for the nhb path.
                # hybrid: at large N the z-path delivers vv after uu, so the
                # v-pairs would serialize into the head — use the direct-h
                # route there (sub on cycle); the split route wins below.
                if t == 0 or N1 >= 165:
                    hmm(pnh, "wnh", True)
                    hmm(pr, "wrh", False)
                    hmm(pz, "wzh", False)
                else:
                    vmm(pnh, "vnh", True)
                    vmm(pr, "vrh", False)
                    vmm(pz, "vzh", False)
                    umm(pnh, "wnh", False, True)
                    umm(pr, "wrh", False, True)
                    umm(pz, "wzh", False, True)

                # scalar engine: sigmoid(r) leads; sigmoid(z) fills the gap
                rs = spool.tile([128, HALF], bf16, tag="rs")
                nc.scalar.activation(rs[:, 0:N1], pr[:, 0:N1], AF.Sigmoid,
                                     bias=biases["br"][:, 0:1], scale=1.0)
                zs = spool.tile([128, HALF], bf16, tag="zs")
                nc.scalar.activation(zs[:, 0:N1], pz[:, 0:N1], AF.Sigmoid,
                                     bias=biases["bz"][:, 0:1], scale=1.0)

                # psum evacuations (off-chain, bf16) + fast on-chain m1/a1
                nhb = spool.tile([128, HALF], bf16, tag="nhb")
                nc.vector.tensor_scalar_add(nhb[:, 0:N1], pnh[:, 0:N1],
                                            biases["bhn"][:, 0:1])
                m1 = spool.tile([128, HALF], bf16, tag="m1")
                nc.vector.tensor_mul(m1[:, 0:N1], rs[:, 0:N1], nhb[:, 0:N1])
                # nib: evacuate pni to bf16 so the on-cycle a1 is a 2x-mode
                # bf16 op instead of a 1x psum read (~120ns cheaper)
                # nib split in two half-width ops: it runs ~2 steps early in
                # whatever vector idle gap the greedy scheduler finds, and a
                # full-width op wedged into the zw->vv gap was stalling uu.
                nib = spool.tile([128, HALF], bf16, tag="nib")
                nh1 = (N1 + 1) // 2 if N1 >= 150 else N1
                nc.vector.tensor_scalar_add(nib[:, 0:nh1], pni[:, 0:nh1], 0.0)
                if nh1 < N1:
                    nc.vector.tensor_scalar_add(nib[:, nh1:N1],
                                                pni[:, nh1:N1], 0.0)
                a1 = spool.tile([128, HALF], bf16, tag="a1")
                nc.vector.tensor_add(a1[:, 0:N1], m1[:, 0:N1], nib[:, 0:N1])
                nt = spool.tile([128, HALF], bf16, tag="nt")
                nc.scalar.activation(nt[:, 0:N1], a1[:, 0:N1], AF.Tanh,
                                     bias=biases["bin"][:, 0:1], scale=1.0)

                # z path off-chain: zw, then vv = (zw-1) (.) h. At large N
                # the GpSimd zw (~2ns/col) lands so late that vv overruns
                # into uu's vector slot (measured the->uu stalls); run zw on
                # the vector engine there instead (~0.5ns/col, fits the
                # a1->tanh idle window). Small N keeps zw on GpSimd to keep
                # the vector queue light.
                zw = spool.tile([128, HALF], bf16, tag="zw")
                if N1 >= 110:
                    nc.vector.tensor_mul(zw[:, 0:N1], zs[:, 0:N1],
                                         pw_cur[:, s, 0:N1])
                else:
                    nc.gpsimd.tensor_mul(zw[:, 0:N1], zs[:, 0:N1],
                                         pw_cur[:, s, 0:N1])
                vv = spool.tile([128, HALF], bf16, tag="vv")
                nc.vector.scalar_tensor_tensor(
                    out=vv[:, 0:N1], in0=zw[:, 0:N1], scalar=1.0,
                    in1=hb_cur[:, s, 0:N1], op0=ALU.subtract, op1=ALU.mult)

                uu = spool.tile([128, HALF], bf16, tag="uu")
                nc.vector.tensor_mul(uu[:, 0:N1], zw[:, 0:N1], nt[:, 0:N1])
                hdst = hb_cur if s == 0 else hb_nxt
                nc.vector.tensor_sub(hdst[:, 1 - s, 0:N1], uu[:, 0:N1],
                                     vv[:, 0:N1])

                if s == 0:
                    wide = int(N1s[2 * p - 1]) if p > 0 else int(N1s[0])
                    nc.sync.dma_start(out=op_d[p, :, 0:2, 0:wide],
                                      in_=hb_cur[:, 0:2, 0:wide])
                if s == 1:
                    hb_cur, x_cur, pw_cur = hb_nxt, x_nxt, pw_nxt
                uu_prev, vv_prev = uu, vv

            vlast = int(N1s[T - 1])
            nc.sync.dma_start(out=op_d[P, :, 0:1, 0:vlast],
                              in_=hb_cur[:, 0:1, 0:vlast])

    nc.compile()
    return nc


def kernel(x, weights, lengths, W_ih, W_hh, b_ih, b_hh):
    global LAST_RESULT
    x = np.asarray(x, dtype=np.float32)
    weights = np.asarray(weights, dtype=np.float32)
    lengths = np.asarray(lengths, dtype=np.int32)
    W_ih = np.asarray(W_ih, dtype=np.float32)
    W_hh = np.asarray(W_hh, dtype=np.float32)
    b_ih = np.asarray(b_ih, dtype=np.float32)
    b_hh = np.asarray(b_hh, dtype=np.float32)

    counts = (lengths[None, :] > np.arange(T)[:, None]).sum(axis=1)  # [T]
    v = -(-counts // NCORES)
    N1s = (v + 1) // 2
    N2s = v // 2

    # x: [T,B,D] -> per-core [P, 128, 2, HALF] (feat-major, halves stacked,
    # two steps per block) — identical to the original kernel's layout
    xr = x.reshape(T, BC, NCORES, D)            # [t, j, k, d], b = 8j+k
    xr = xr.transpose(2, 0, 3, 1)               # [k, t, d, j]
    xr = xr.reshape(NCORES, T, D, HALF, 2)      # j = 2c + half
    xr = xr.transpose(0, 1, 4, 2, 3)            # [k, t, half, d, c]
    x8 = xr.reshape(NCORES, T, 128, HALF).astype(ml_dtypes.bfloat16)
    x8 = np.ascontiguousarray(
        x8.reshape(NCORES, P, 2, 128, HALF).transpose(0, 1, 3, 2, 4))

    wr = weights[:, :, 0].reshape(T, BC, NCORES)   # [t, j, k]
    wr = wr.transpose(2, 0, 1)                     # [k, t, j]
    wr = wr.reshape(NCORES, T, HALF, 2).transpose(0, 1, 3, 2)  # [k,t,half,c]
    w8 = np.broadcast_to(wr[:, :, :, None, :],
                         (NCORES, T, 2, 64, HALF))  # [k, t, half, d, c]
    w8 = w8.reshape(NCORES, T, 128, HALF)
    w8 = np.ascontiguousarray(
        w8.reshape(NCORES, P, 2, 128, HALF).transpose(0, 1, 3, 2, 4)
    ).astype(ml_dtypes.bfloat16)

    # wblob: six [128, 64] tiles (each = [Wg.T; Wg.T] halves-stacked) packed
    # side by side; bblob: the four [128, 1] biases packed side by side.
    wtiles = [np.vstack([wg.T, wg.T]) for wg in
              (W_ih[0:64], W_ih[64:128], W_ih[128:192],
               W_hh[0:64], W_hh[64:128], W_hh[128:192],
               -W_hh[0:64], -W_hh[64:128], -W_hh[128:192])]
    wblob = np.ascontiguousarray(
        np.concatenate(wtiles, axis=1)).astype(ml_dtypes.bfloat16)
    b_r = (b_ih[0:64] + b_hh[0:64]).astype(np.float32)
    b_z = (b_ih[64:128] + b_hh[64:128]).astype(np.float32)
    b_hn = b_hh[128:192].astype(np.float32)
    b_in = b_ih[128:192].astype(np.float32)
    bblob = np.stack([np.tile(b_r, 2), np.tile(b_z, 2),
                      np.tile(b_hn, 2), np.tile(b_in, 2)],
                     axis=1).astype(np.float32)  # [128, 4]
    nc = _build_program(N1s, N2s)

    in_maps = []
    for k in range(NCORES):
        m = {"x": x8[k], "w": w8[k], "wblob": wblob, "bblob": bblob}
        in_maps.append(m)

    trace = bool(os.environ.get("AUGRU_TRACE"))
    tmpdir = os.environ.get("AUGRU_PROF_DIR") or None
    res = run_bass_kernel_spmd(nc, in_maps, list(range(NCORES)), trace=trace,
                               tmpdir=tmpdir)
    LAST_RESULT = res

    # op: [k, P+1, 128, 2, HALF]; row t' = 2p+s holds h_{t'} = out[t'-1]
    outs = np.stack([np.asarray(res.results[k]["op"]) for k in range(NCORES)])
    o = outs.astype(np.float32)                       # [k, p, 128, s, c]
    o = o.transpose(1, 3, 0, 2, 4)                    # [p, s, k, 128, c]
    o = o.reshape(2 * (P + 1), NCORES, 2, H, HALF)    # [t', k, half, d, c]
    o = o.transpose(0, 4, 2, 1, 3)                    # [t', c, half, k, d]
    o = o.reshape(2 * (P + 1), B, H)[1:T + 1]         # j = 2c+half, b = 8j+k
    mask = (np.arange(T)[:, None] < lengths[None, :])
    o = np.where(mask[:, :, None], o, np.float32(0.0)).astype(np.float32)
    return o



# revision 44
# speedup vs baseline: 1.0154x; 1.0053x over previous
"""AUGRU Trainium2 kernel v6 — v3 chain + packed-constant startup.

Same batch/half layout as the original kernel (b = 8j+k strided across
cores; j = 2c+half interleaved halves stacked on partitions; all on-chip
tensors [128, cols<=256]).

v6 delta vs v3: the 16 serialized constant DMAs (6 weights x 2 halves +
4 biases, ~645ns each on the one sync queue; ~10.3us before the first
matmul) are packed into two blobs on the host -- wblob [128, 384] bf16
(six [Wg.T; Wg.T] tiles side by side) and bblob [128, 4] f32 -- loaded
with two DMAs ordered ahead of the pair-0 x/w loads. The steady-state
loop's instruction stream is unchanged (weight/bias APs are views into
the blob tiles), saving ~8us of startup.

Chain shaves vs the original baseline (from v3):
  - psum gate outputs (nh, ni) are evacuated to bf16 SBUF off the critical
    path (nhb = nh + bhn via tensor_scalar_add; nib = copy), so the on-chain
    m1/a1 become 2x-mode bf16 tensor ops (~165ns instead of 351/380).
  - state is bf16 end-to-end: single h' sub, no f32 shadow, output DMA'd
    as bf16 from the h-state pair tiles (host masks + converts).
  - attention weights w are host-broadcast to [128, HALF] bf16 and DMA'd
    (no GpSimd DIRECT2D); zw multiply runs on GpSimd off-chain.
  - x-side matmuls sit ahead of the h-side ones in the PE queue, so they
    execute during the previous step's elementwise tail.
"""

import os
import ml_dtypes
import numpy as np

import concourse.bass as bass
import concourse.bacc as bacc
import concourse.mybir as mybir
from concourse.tile import TileContext
from concourse.bass_utils import run_bass_kernel_spmd

T, B, D, H = 200, 4096, 64, 64
NCORES = 8
BC = B // NCORES  # 512 batch rows per core
HALF = BC // 2    # 256 columns per half
P = T // 2        # step pairs

LAST_RESULT = None

f32 = mybir.dt.float32
bf16 = mybir.dt.bfloat16
AF = mybir.ActivationFunctionType
ALU = mybir.AluOpType


def _build_program(N1s, N2s):
    nc = bacc.Bacc()

    x_d = nc.declare_dram_parameter("x", [P, 128, 2, HALF], bf16, isOutput=False)
    w_d = nc.declare_dram_parameter("w", [P, 128, 2, HALF], bf16, isOutput=False)
    # all six weight tiles packed [128, 6*64] (halves pre-stacked on the
    # host) + the four biases packed [128, 4]: two startup DMAs, not 16
    wblob_d = nc.declare_dram_parameter("wblob", [128, 9 * H], bf16,
                                        isOutput=False)
    bblob_d = nc.declare_dram_parameter("bblob", [128, 4], f32, isOutput=False)
    op_d = nc.declare_dram_parameter("op", [P + 1, 128, 2, HALF], bf16,
                                     isOutput=True)

    with TileContext(nc) as tc:
        with (
            tc.tile_pool(name="const", bufs=1) as cpool,
            tc.tile_pool(name="hb", bufs=3) as hbpool,
            tc.tile_pool(name="xin", bufs=3) as xpool,
            tc.tile_pool(name="win", bufs=3) as wpool,
            tc.tile_pool(name="work", bufs=2) as spool,
            tc.tile_pool(name="ps", bufs=2, space="PSUM") as ppool,
        ):
            wv0 = int(N1s[0])
            hb_cur = hbpool.tile([128, 2, HALF], bf16, tag="hb")
            nc.vector.memset(hb_cur[:, 0, :], 0.0)
            wtile = cpool.tile([128, 9 * H], bf16, tag="wblob")
            nc.sync.dma_start(out=wtile[:, :], in_=wblob_d[:, :])
            x_cur = xpool.tile([128, 2, HALF], bf16, tag="x")
            nc.sync.dma_start(out=x_cur[:, :, 0:wv0], in_=x_d[0, :, :, 0:wv0])
            pw_cur = wpool.tile([128, 2, HALF], bf16, tag="pw")
            nc.sync.dma_start(out=pw_cur[:, :, 0:wv0], in_=w_d[0, :, :, 0:wv0])
            hb_nxt = x_nxt = pw_nxt = None

            btile = cpool.tile([128, 4], f32, tag="bblob")
            nc.sync.dma_start(out=btile[:, :], in_=bblob_d[:, :])
            wts = {
                name: wtile[:, i * H:(i + 1) * H]
                for i, name in enumerate(
                    ["wrx", "wzx", "wnx", "wrh", "wzh", "wnh",
                     "vrh", "vzh", "vnh"])
            }
            biases = {
                name: btile[:, i:i + 1]
                for i, name in enumerate(["br", "bz", "bhn", "bin"])
            }

            for t in range(T):
                N1 = int(N1s[t])
                N2 = int(N2s[t])
                if N1 == 0:
                    break
                p, s = divmod(t, 2)

                if s == 0:
                    hb_nxt = hbpool.tile([128, 2, HALF], bf16, tag="hb")
                    x_nxt = xpool.tile([128, 2, HALF], bf16, tag="x")
                    pw_nxt = wpool.tile([128, 2, HALF], bf16, tag="pw")
                    if p + 1 < P:
                        wvn = int(N1s[2 * (p + 1)])
                        if wvn > 0:
                            nc.sync.dma_start(out=x_nxt[:, :, 0:wvn],
                                              in_=x_d[p + 1, :, :, 0:wvn])
                            nc.sync.dma_start(out=pw_nxt[:, :, 0:wvn],
                                              in_=w_d[p + 1, :, :, 0:wvn])

                pr = ppool.tile([128, HALF], f32, tag="pr")
                pz = ppool.tile([128, HALF], f32, tag="pz")
                # pni/pnh as SEPARATE tiles (not two regions of one tile):
                # dependency tracking is tile-granular, so a shared tile made
                # nib wait on the pnh h-side matmuls and blocked m1's slot.
                pni = ppool.tile([128, HALF], f32, tag="pni")
                pnh = ppool.tile([128, HALF], f32, tag="pnh")

                def xmm(psum, wx, stop):
                    nc.tensor.matmul(psum[0:64, 0:N1], lhsT=wts[wx][0:64, :],
                                     rhs=x_cur[0:64, s, 0:N1], start=True,
                                     stop=stop)
                    if N2 > 0:
                        nc.tensor.matmul(psum[64:128, 0:N2],
                                         lhsT=wts[wx][64:128, :],
                                         rhs=x_cur[64:128, s, 0:N2],
                                         start=True, stop=stop)

                def hmm(psum, wh, start):
                    nc.tensor.matmul(psum[0:64, 0:N1], lhsT=wts[wh][0:64, :],
                                     rhs=hb_cur[0:64, s, 0:N1], start=start,
                                     stop=True)
                    if N2 > 0:
                        nc.tensor.matmul(psum[64:128, 0:N2],
                                         lhsT=wts[wh][64:128, :],
                                         rhs=hb_cur[64:128, s, 0:N2],
                                         start=start, stop=True)

                def umm(psum, wh, start, stop):
                    nc.tensor.matmul(psum[0:64, 0:N1], lhsT=wts[wh][0:64, :],
                                     rhs=uu_prev[0:64, 0:N1], start=start,
                                     stop=stop)
                    if N2 > 0:
                        nc.tensor.matmul(psum[64:128, 0:N2],
                                         lhsT=wts[wh][64:128, :],
                                         rhs=uu_prev[64:128, 0:N2],
                                         start=start, stop=stop)

                def vmm(psum, wh, start):
                    nc.tensor.matmul(psum[0:64, 0:N1], lhsT=wts[wh][0:64, :],
                                     rhs=vv_prev[0:64, 0:N1], start=start,
                                     stop=False)
                    if N2 > 0:
                        nc.tensor.matmul(psum[64:128, 0:N2],
                                         lhsT=wts[wh][64:128, :],
                                         rhs=vv_prev[64:128, 0:N2],
                                         start=start, stop=False)

                # x-side first: these run during the previous step's tail
                xmm(pr, "wrx", False)
                xmm(pz, "wzx", False)
                xmm(pni, "wnx", True)
                # h-side on (uu, vv) of the previous step: h = uu - vv is
                # distributed over the matmuls (negated v-side weights), so
                # `sub` leaves the critical cycle. vv lands before uu
                # (measured), so the v-pairs stream on the PE just ahead of
                # the u-pairs; u-side keeps pnh-first for the nhb path.
                # hybrid: at large N the z-path delivers vv after uu, so the
                # v-pairs would serialize into the head — use the direct-h
                # route there (sub on cycle); the split route wins below.
                if t == 0 or N1 >= 180:
                    hmm(pnh, "wnh", True)
                    hmm(pr, "wrh", False)
                    hmm(pz, "wzh", False)
                else:
                    vmm(pnh, "vnh", True)
                    vmm(pr, "vrh", False)
                    vmm(pz, "vzh", False)
                    umm(pnh, "wnh", False, True)
                    umm(pr, "wrh", False, True)
                    umm(pz, "wzh", False, True)

                # scalar engine: sigmoid(r) leads; sigmoid(z) fills the gap
                rs = spool.tile([128, HALF], bf16, tag="rs")
                nc.scalar.activation(rs[:, 0:N1], pr[:, 0:N1], AF.Sigmoid,
                                     bias=biases["br"][:, 0:1], scale=1.0)
                zs = spool.tile([128, HALF], bf16, tag="zs")
                nc.scalar.activation(zs[:, 0:N1], pz[:, 0:N1], AF.Sigmoid,
                                     bias=biases["bz"][:, 0:1], scale=1.0)

                # psum evacuations (off-chain, bf16) + fast on-chain m1/a1
                nhb = spool.tile([128, HALF], bf16, tag="nhb")
                nc.vector.tensor_scalar_add(nhb[:, 0:N1], pnh[:, 0:N1],
                                            biases["bhn"][:, 0:1])
                m1 = spool.tile([128, HALF], bf16, tag="m1")
                nc.vector.tensor_mul(m1[:, 0:N1], rs[:, 0:N1], nhb[:, 0:N1])
                # nib: evacuate pni to bf16 so the on-cycle a1 is a 2x-mode
                # bf16 op instead of a 1x psum read (~120ns cheaper)
                # nib split in two half-width ops: it runs ~2 steps early in
                # whatever vector idle gap the greedy scheduler finds, and a
                # full-width op wedged into the zw->vv gap was stalling uu.
                nib = spool.tile([128, HALF], bf16, tag="nib")
                nh1 = (N1 + 1) // 2 if N1 >= 150 else N1
                nc.vector.tensor_scalar_add(nib[:, 0:nh1], pni[:, 0:nh1], 0.0)
                if nh1 < N1:
                    nc.vector.tensor_scalar_add(nib[:, nh1:N1],
                                                pni[:, nh1:N1], 0.0)
                a1 = spool.tile([128, HALF], bf16, tag="a1")
                nc.vector.tensor_add(a1[:, 0:N1], m1[:, 0:N1], nib[:, 0:N1])
                nt = spool.tile([128, HALF], bf16, tag="nt")
                nc.scalar.activation(nt[:, 0:N1], a1[:, 0:N1], AF.Tanh,
                                     bias=biases["bin"][:, 0:1], scale=1.0)

                # z path off-chain: zw, then vv = (zw-1) (.) h. At large N
                # the GpSimd zw (~2ns/col) lands so late that vv overruns
                # into uu's vector slot (measured the->uu stalls); run zw on
                # the vector engine there instead (~0.5ns/col, fits the
                # a1->tanh idle window). Small N keeps zw on GpSimd to keep
                # the vector queue light.
                zw = spool.tile([128, HALF], bf16, tag="zw")
                if N1 >= 110:
                    nc.vector.tensor_mul(zw[:, 0:N1], zs[:, 0:N1],
                                         pw_cur[:, s, 0:N1])
                else:
                    nc.gpsimd.tensor_mul(zw[:, 0:N1], zs[:, 0:N1],
                                         pw_cur[:, s, 0:N1])
                vv = spool.tile([128, HALF], bf16, tag="vv")
                nc.vector.scalar_tensor_tensor(
                    out=vv[:, 0:N1], in0=zw[:, 0:N1], scalar=1.0,
                    in1=hb_cur[:, s, 0:N1], op0=ALU.subtract, op1=ALU.mult)

                uu = spool.tile([128, HALF], bf16, tag="uu")
                nc.vector.tensor_mul(uu[:, 0:N1], zw[:, 0:N1], nt[:, 0:N1])
                hdst = hb_cur if s == 0 else hb_nxt
                nc.vector.tensor_sub(hdst[:, 1 - s, 0:N1], uu[:, 0:N1],
                                     vv[:, 0:N1])

                if s == 0:
                    wide = int(N1s[2 * p - 1]) if p > 0 else int(N1s[0])
                    nc.sync.dma_start(out=op_d[p, :, 0:2, 0:wide],
                                      in_=hb_cur[:, 0:2, 0:wide])
                if s == 1:
                    hb_cur, x_cur, pw_cur = hb_nxt, x_nxt, pw_nxt
                uu_prev, vv_prev = uu, vv

            vlast = int(N1s[T - 1])
            nc.sync.dma_start(out=op_d[P, :, 0:1, 0:vlast],
                              in_=hb_cur[:, 0:1, 0:vlast])

    nc.compile()
    return nc


def kernel(x, weights, lengths, W_ih, W_hh, b_ih, b_hh):
    global LAST_RESULT
    x = np.asarray(x, dtype=np.float32)
    weights = np.asarray(weights, dtype=np.float32)
    lengths = np.asarray(lengths, dtype=np.int32)
    W_ih = np.asarray(W_ih, dtype=np.float32)
    W_hh = np.asarray(W_hh, dtype=np.float32)
    b_ih = np.asarray(b_ih, dtype=np.float32)
    b_hh = np.asarray(b_hh, dtype=np.float32)

    counts = (lengths[None, :] > np.arange(T)[:, None]).sum(axis=1)  # [T]
    v = -(-counts // NCORES)
    N1s = (v + 1) // 2
    N2s = v // 2

    # x: [T,B,D] -> per-core [P, 128, 2, HALF] (feat-major, halves stacked,
    # two steps per block) — identical to the original kernel's layout
    xr = x.reshape(T, BC, NCORES, D)            # [t, j, k, d], b = 8j+k
    xr = xr.transpose(2, 0, 3, 1)               # [k, t, d, j]
    xr = xr.reshape(NCORES, T, D, HALF, 2)      # j = 2c + half
    xr = xr.transpose(0, 1, 4, 2, 3)            # [k, t, half, d, c]
    x8 = xr.reshape(NCORES, T, 128, HALF).astype(ml_dtypes.bfloat16)
    x8 = np.ascontiguousarray(
        x8.reshape(NCORES, P, 2, 128, HALF).transpose(0, 1, 3, 2, 4))

    wr = weights[:, :, 0].reshape(T, BC, NCORES)   # [t, j, k]
    wr = wr.transpose(2, 0, 1)                     # [k, t, j]
    wr = wr.reshape(NCORES, T, HALF, 2).transpose(0, 1, 3, 2)  # [k,t,half,c]
    w8 = np.broadcast_to(wr[:, :, :, None, :],
                         (NCORES, T, 2, 64, HALF))  # [k, t, half, d, c]
    w8 = w8.reshape(NCORES, T, 128, HALF)
    w8 = np.ascontiguousarray(
        w8.reshape(NCORES, P, 2, 128, HALF).transpose(0, 1, 3, 2, 4)
    ).astype(ml_dtypes.bfloat16)

    # wblob: six [128, 64] tiles (each = [Wg.T; Wg.T] halves-stacked) packed
    # side by side; bblob: the four [128, 1] biases packed side by side.
    wtiles = [np.vstack([wg.T, wg.T]) for wg in
              (W_ih[0:64], W_ih[64:128], W_ih[128:192],
               W_hh[0:64], W_hh[64:128], W_hh[128:192],
               -W_hh[0:64], -W_hh[64:128], -W_hh[128:192])]
    wblob = np.ascontiguousarray(
        np.concatenate(wtiles, axis=1)).astype(ml_dtypes.bfloat16)
    b_r = (b_ih[0:64] + b_hh[0:64]).astype(np.float32)
    b_z = (b_ih[64:128] + b_hh[64:128]).astype(np.float32)
    b_hn = b_hh[128:192].astype(np.float32)
    b_in = b_ih[128:192].astype(np.float32)
    bblob = np.stack([np.tile(b_r, 2), np.tile(b_z, 2),
                      np.tile(b_hn, 2), np.tile(b_in, 2)],
                     axis=1).astype(np.float32)  # [128, 4]
    nc = _build_program(N1s, N2s)

    in_maps = []
    for k in range(NCORES):
        m = {"x": x8[k], "w": w8[k], "wblob": wblob, "bblob": bblob}
        in_maps.append(m)

    trace = bool(os.environ.get("AUGRU_TRACE"))
    tmpdir = os.environ.get("AUGRU_PROF_DIR") or None
    res = run_bass_kernel_spmd(nc, in_maps, list(range(NCORES)), trace=trace,
                               tmpdir=tmpdir)
    LAST_RESULT = res

    # op: [k, P+1, 128, 2, HALF]; row t' = 2p+s holds h_{t'} = out[t'-1]
    outs = np.stack([np.asarray(res.results[k]["op"]) for k in range(NCORES)])
    o = outs.astype(np.float32)                       # [k, p, 128, s, c]
    o = o.transpose(1, 3, 0, 2, 4)                    # [p, s, k, 128, c]
    o = o.reshape(2 * (P + 1), NCORES, 2, H, HALF)    # [t', k, half, d, c]
    o = o.transpose(0, 4, 2, 1, 3)                    # [t', c, half, k, d]
    o = o.reshape(2 * (P + 1), B, H)[1:T + 1]         # j = 2c+half, b = 8j+k
    mask = (np.arange(T)[:, None] < lengths[None, :])
    o = np.where(mask[:, :, None], o, np.float32(0.0)).astype(np.float32)
    return o

